# revision 32
# baseline (speedup 1.0000x reference)
"""EGNN (N=384, D=3, H=128, L=4) Bass kernel for 8 TRN2 NeuronCores.

Sharding: rows of the N x N edge grid split across 8 cores (48 rows each).
Each core holds full h; per layer it computes its row-block of the coord/edge
MLPs and row-sums (phi-weighted coordinate update, gated message sum), then
AllGathers the per-node x updates and msum rows. The h-node MLP is computed
redundantly on every core (384 cols, tiny). The embedding row-sum of
emb_w (49152 x 384, the dominant HBM traffic) is sharded 1/8 per core.

Self-contained: shapes hardcoded, inputs are the full unsharded arrays.
"""
import os
import numpy as np

DEBUG = os.environ.get("KDEBUG", "0") == "1"
BF16MLP = os.environ.get("KBF16", "0") == "1"

N, D, H, L = 384, 3, 128, 4
NC = 8
NI = N // NC          # 48 rows per core
NJ = N                # 384 cols
G = 2                 # i-rows per group
NGRP = NI // G        # 24 groups
EMB_ROWS = N * H // NC  # 6144 emb_w rows per core

_cache = {}


def _build_nc():
    import concourse.bass as bass
    import concourse.bacc as bacc
    import concourse.tile as tile
    from concourse import mybir

    F32 = mybir.dt.float32
    AF = mybir.ActivationFunctionType
    OP = mybir.AluOpType

    nc = bacc.Bacc(None, target_bir_lowering=False)
    F32R = mybir.dt.float32r

    def RMM(out, lhsT, rhs, **kw):
        nc.tensor.matmul(out, lhsT=lhsT.bitcast(F32R), rhs=rhs.bitcast(F32R), **kw)

    def P(name, shape):
        return nc.declare_dram_parameter(name, list(shape), F32, isOutput=False)

    # per-core inputs
    embw = P("embw", (EMB_ROWS, NJ))
    embbT = P("embbT", (H, NI))
    x0my = P("x0my", (NI, D))
    maskc = P("maskc", (NI, NJ))
    eyec = P("eyec", (NI, NJ))
    # shared inputs
    x0rows = P("x0rows", (1, D * NJ))
    c1hiT = P("c1hiT", (L, H, H))
    c1hjT = P("c1hjT", (L, H, H))
    c1drep = P("c1drep", (L, 16 * H))
    cb1 = P("cb1", (H, L))
    c2T = P("c2T", (L, H, H))
    cb2 = P("cb2", (H, L))
    c3w = P("c3w", (L, H, 2 * NI - 1))
    cb3c = P("cb3c", (NI, L))
    e1hiT = P("e1hiT", (L - 1, H, H))
    e1hjT = P("e1hjT", (L - 1, H, H))
    e1drep = P("e1drep", (L - 1, 16 * H))
    ones6k = P("ones6k", (1, 16 * NJ))
    eb1 = P("eb1", (H, L - 1))
    e2T = P("e2T", (L - 1, H, H))
    eb2 = P("eb2", (H, L - 1))
    attw = P("attw", (L - 1, H, 2 * NI - 1))
    nw1hT = P("nw1hT", (L - 1, H, H))
    nw1mT = P("nw1mT", (L - 1, H, H))
    nb1 = P("nb1", (H, L - 1))
    nw2T = P("nw2T", (L - 1, H, H))
    nb2 = P("nb2", (H, L - 1))
    ones128 = P("ones128", (1, H))

    o_x = nc.declare_dram_parameter("o_x", [N, D], F32, isOutput=True)
    dbg = {}
    if DEBUG:
        for nm, shp in [("h0", (H, NJ)), ("d2", (NI, NJ)), ("u", (NI, NJ)),
                        ("t1c", (H, G * NJ)), ("phis", (NI, NJ)),
                        ("msum", (H, NI)), ("x1", (NI, D)), ("h1", (H, NJ)),
                        ("gmask", (NI, NJ))]:
            dbg[nm] = nc.declare_dram_parameter("dbg_" + nm, list(shp), F32,
                                                isOutput=True)

    # collective bounce buffers
    hag_in = nc.dram_tensor("hag_in", [H, NI], F32)
    hag_out = nc.dram_tensor("hag_out", [NC * H, NI], F32, addr_space="Shared")
    xag_in = [nc.dram_tensor(f"xag_in{l}", [D, NI], F32) for l in range(L - 1)]
    xag_in.append(nc.dram_tensor(f"xag_in3", [NI, D], F32))
    xag_out = [nc.dram_tensor(f"xag_out{l}", [NC, D, NI], F32, addr_space="Shared")
               for l in range(L - 1)]
    xag_out.append(nc.dram_tensor(f"xag_out3", [N, D], F32, addr_space="Shared"))
    mag_in = [nc.dram_tensor(f"mag_in{l}", [H, NI], F32) for l in range(L - 1)]
    mag_out = [nc.dram_tensor(f"mag_out{l}", [NC * H, NI], F32, addr_space="Shared")
               for l in range(L - 1)]
    rg = [list(range(NC))]

    with tile.TileContext(nc) as tc:
        with (
            tc.tile_pool(name="consts", bufs=1) as consts,
            tc.tile_pool(name="embp", bufs=2) as embp,
            tc.tile_pool(name="work", bufs=2) as work,
            tc.tile_pool(name="slab", bufs=1) as slabp,
            tc.tile_pool(name="cp1", bufs=1) as cp1,
            tc.tile_pool(name="cp2", bufs=2) as cp2,
            tc.tile_pool(name="rows", bufs=2) as rowsp,
            tc.tile_pool(name="mgp", bufs=1) as mgp,
            tc.tile_pool(name="ps_mlp", bufs=3, space="PSUM") as ps_mlp,
            tc.tile_pool(name="ps_sm", bufs=2, space="PSUM") as ps_sm,
        ):
            # ---- load constants ----
            def load(pname, ap_in, shape, rnd=False):
                t = consts.tile(list(shape), F32, tag=pname)
                o = t[:].bitcast(F32R) if rnd else t
                nc.sync.dma_start(out=o, in_=ap_in.bitcast(F32R) if rnd else ap_in)
                return t

            c1hiT_sb = load("c1hiT", c1hiT.rearrange("l p x -> p l x"), (H, L, H),
                            rnd=True)
            c1hjT_sb = load("c1hjT", c1hjT.rearrange("l p x -> p l x"), (H, L, H),
                            rnd=True)
            c2T_sb = load("c2T", c2T.rearrange("l p x -> p l x"), (H, L, H),
                          rnd=True)
            c3w_sb = load("c3w", c3w.rearrange("l p x -> p l x"),
                          (H, L, 2 * NI - 1), rnd=True)

            cb1_sb = load("cb1", cb1[:], (H, L))
            cb2_sb = load("cb2", cb2[:], (H, L))
            cb3c_sb = load("cb3c", cb3c[:], (NI, L))
            e1hiT_sb = load("e1hiT", e1hiT.rearrange("l p x -> p l x"),
                            (H, L - 1, H), rnd=True)
            e1hjT_sb = load("e1hjT", e1hjT.rearrange("l p x -> p l x"),
                            (H, L - 1, H), rnd=True)
            e2T_sb = load("e2T", e2T.rearrange("l p x -> p l x"), (H, L - 1, H),
                          rnd=True)
            attw_sb = load("attw", attw.rearrange("l p x -> p l x"),
                           (H, L - 1, 2 * NI - 1), rnd=True)

            eb1_sb = load("eb1", eb1[:], (H, L - 1))
            eb2_sb = load("eb2", eb2[:], (H, L - 1))
            nw1hT_sb = load("nw1hT", nw1hT.rearrange("l p x -> p l x"),
                            (H, L - 1, H), rnd=True)
            nw1mT_sb = load("nw1mT", nw1mT.rearrange("l p x -> p l x"),
                            (H, L - 1, H), rnd=True)
            nw2T_sb = load("nw2T", nw2T.rearrange("l p x -> p l x"),
                           (H, L - 1, H), rnd=True)
            nb1_sb = load("nb1", nb1[:], (H, L - 1))
            nb2_sb = load("nb2", nb2[:], (H, L - 1))
            maskc_sb = load("maskc", maskc[:], (NI, NJ))
            eyec_sb = load("eyec", eyec[:], (NI, NJ))
            embbT_sb = load("embbT", embbT[:], (H, NI))
            ones_sb = load("ones128", ones128[:], (1, H), rnd=True)
            x0my_sb = load("x0my", x0my[:], (NI, D))

            BF16 = mybir.dt.bfloat16
            MLPDT = BF16 if BF16MLP else F32
            def MLPV(ap):
                # view for matmul operands of MLP-pass tiles
                return ap if BF16MLP else ap.bitcast(F32R)

            if BF16MLP:
                def tobf(t, pname):
                    b = consts.tile(list(t.shape), BF16, tag=pname + "_bf")
                    nc.vector.tensor_copy(b, t)
                    return b
                c1hjT_w = tobf(c1hjT_sb, "c1hjT")
                c2T_w = tobf(c2T_sb, "c2T")
                c3w_w = tobf(c3w_sb, "c3w")
                e1hjT_w = tobf(e1hjT_sb, "e1hjT")
                e2T_w = tobf(e2T_sb, "e2T")
                attw_w = tobf(attw_sb, "attw")
                ones_w = tobf(ones_sb, "ones128")
            else:
                c1hjT_w, c2T_w = c1hjT_sb, c2T_sb
                c3w_w, e1hjT_w = c3w_sb, e1hjT_sb
                e2T_w, attw_w, ones_w = e2T_sb, attw_sb, ones_sb

            d2ones = consts.tile([66, 16 * NJ], F32, tag="d2ones")
            combC = consts.tile([66, 16 * H], F32, tag="combC")
            combE = consts.tile([66, 16 * H], F32, tag="combE")
            for k in range(3):
                nc.sync.dma_start(out=d2ones[32 * k + 1:32 * k + 2, :]
                                  .bitcast(F32R), in_=ones6k[:].bitcast(F32R))

            # ---- phase 0: embedding row-sum (memory bound) ----
            hT0 = cp1.tile([H, NI], F32, tag="hT0")
            EB = 4  # nodes per embedding DMA
            for t in range(NI // EB):
                et = embp.tile([H, EB, NJ], F32, tag="embt")
                nc.sync.dma_start(
                    out=et,
                    in_=embw[t * EB * H:(t + 1) * EB * H, :]
                    .rearrange("(a p) j -> p a j", p=H))
                with nc.allow_low_precision(reason="f32r storage, f32 accum"):
                    nc.vector.tensor_reduce(
                        out=hT0[:, t * EB:(t + 1) * EB].bitcast(F32R), in_=et,
                        axis=mybir.AxisListType.X, op=OP.add,
                    )
            nc.vector.tensor_tensor(out=hT0[:].bitcast(F32R), in0=hT0,
                                    in1=embbT_sb, op=OP.add)
            nc.sync.dma_start(out=hag_in[:], in_=hT0)
            nc.gpsimd.collective_compute(
                "AllGather", OP.bypass, replica_groups=rg,
                ins=[hag_in[:]], outs=[hag_out[:]],
            )
            h_T = cp2.tile([H, NJ], F32, tag="hT")
            for r in range(NC):
                nc.sync.dma_start(out=h_T[:, r * NI:(r + 1) * NI].bitcast(F32R),
                                  in_=hag_out[r * H:(r + 1) * H, :].bitcast(F32R))
            h_my = hT0  # this core's own h rows (h_T columns 48c..48c+48)
            if DEBUG:
                nc.sync.dma_start(out=dbg["h0"][:], in_=h_T)

            x_my = x0my_sb

            for l in range(L):
                last = l == L - 1
                # ---- A: coordinate prep ----
                diff = []
                for c in range(D):
                    xb = cp1.tile([NI, NJ], F32, tag=f"xb{c}")
                    if l == 0:
                        bsrc = bass.AP(tensor=x0rows, offset=c * NJ,
                                       ap=[[0, NI], [1, NJ]])
                    else:
                        bsrc = bass.AP(tensor=xag_out[l - 1], offset=c * NI,
                                       ap=[[0, NI], [D * NI, NC], [1, NI]])
                    nc.sync.dma_start(out=xb, in_=bsrc)
                    dc = cp2.tile([NI, NJ], F32, tag=f"diff{c}")
                    nc.vector.tensor_scalar(
                        out=dc, in0=xb, scalar1=x_my[:, c:c + 1], scalar2=None,
                        op0=OP.subtract,
                    )
                    diff.append(dc)
                d2 = cp1.tile([NI, NJ], F32, tag="d2")
                tmp = cp1.tile([NI, NJ], F32, tag="ctmp")
                nc.vector.tensor_tensor(out=d2, in0=diff[0], in1=diff[0], op=OP.mult)
                nc.vector.tensor_tensor(out=tmp, in0=diff[1], in1=diff[1], op=OP.mult)
                nc.vector.tensor_tensor(out=d2, in0=d2, in1=tmp, op=OP.add)
                nc.vector.tensor_tensor(out=tmp, in0=diff[2], in1=diff[2], op=OP.mult)
                nc.vector.tensor_tensor(out=d2, in0=d2, in1=tmp, op=OP.add)
                d2s = cp1.tile([NI, NJ], F32, tag="d2s")
                nc.vector.tensor_tensor(out=d2s, in0=d2, in1=eyec_sb, op=OP.add)
                dn = cp1.tile([NI, NJ], F32, tag="dn")
                nc.scalar.activation(out=dn, in_=d2s, func=AF.Sqrt)
                nc.vector.tensor_scalar(out=dn, in0=dn, scalar1=1.0, scalar2=None,
                                        op0=OP.add)
                u = cp2.tile([NI, NJ], F32, tag="u")
                nc.vector.reciprocal(out=u, in_=dn)

                if DEBUG and l == 0:
                    nc.sync.dma_start(out=dbg["d2"][:], in_=d2)
                    nc.sync.dma_start(out=dbg["u"][:], in_=u)

                # ---- B: edge-grid MLP groups ----
                hT_l = h_T    # h for THIS layer (h_T gets rebound by node MLP)
                hmy_l = h_my
                if BF16MLP:
                    hT_mlp = cp2.tile([H, NJ], BF16, tag="hTb")
                    nc.vector.tensor_copy(hT_mlp, h_T)
                    d2_mlp = cp1.tile([NI, NJ], BF16, tag="d2b")
                    nc.vector.tensor_copy(d2_mlp, d2)
                else:
                    hT_mlp = h_T
                    d2_mlp = d2
                for k in range(3):
                    nc.sync.dma_start(
                        out=d2ones[32 * k:32 * k + 1, :].bitcast(F32R),
                        in_=d2_mlp[16 * k:16 * (k + 1), :].bitcast(F32R))

                def make_comb(hiT, wdrep_l, comb, atag):
                    # A_T = (W1hi @ h_my).T via one matmul; stripe [wd; A_row]
                    # pairs into comb for the K=2 d-pass
                    At_ps = ps_sm.tile([NI, H], F32, tag="sm")
                    RMM(At_ps, lhsT=hmy_l, rhs=hiT, start=True, stop=True)
                    At = cp1.tile([NI, H], F32, tag=atag)
                    nc.vector.tensor_copy(At[:].bitcast(F32R), At_ps)
                    for k in range(3):
                        nc.sync.dma_start(
                            out=comb[32 * k:32 * k + 1, :].bitcast(F32R),
                            in_=wdrep_l.bitcast(F32R))
                        nc.sync.dma_start(
                            out=comb[32 * k + 1:32 * k + 2, :].bitcast(F32R),
                            in_=At[16 * k:16 * (k + 1), :].bitcast(F32R))

                make_comb(c1hiT_sb[:, l, :], c1drep[l:l + 1, :], combC, "Atc")
                if not last:
                    make_comb(e1hiT_sb[:, l, :], e1drep[l:l + 1, :], combE, "Ate")

                def emit_group(comb, b1c, hjT, w2T, b2c, winT, acc_ps,
                               slab, g):
                    pre = ps_mlp.tile([H, G * 512], F32, tag="mlp")

                    def mm_hj(first):
                        for r in range(G):
                            nc.tensor.matmul(
                                pre[:, r * 512:r * 512 + NJ],
                                lhsT=MLPV(hjT),
                                rhs=MLPV(hT_mlp[:, :]),
                                start=first, stop=not first)

                    def mm_d(first):
                        for r in range(G):
                            i = G * g + r
                            kq, iq = i // 16, i % 16
                            nc.tensor.matmul(
                                pre[:, r * 512:r * 512 + NJ],
                                lhsT=comb[32 * kq:32 * kq + 2,
                                          iq * H:(iq + 1) * H].bitcast(F32R),
                                rhs=d2ones[32 * kq:32 * kq + 2,
                                           iq * NJ:(iq + 1) * NJ].bitcast(F32R),
                                start=first, stop=not first)

                    mm_hj(True)
                    mm_d(False)
                    t1 = work.tile([H, G * NJ], MLPDT, tag="t1")
                    nc.scalar.activation(
                        out=(t1[:, :] if BF16MLP else t1[:, :].bitcast(F32R))
                        .rearrange("p (r c) -> p r c", r=G),
                        in_=pre.rearrange("p (r c) -> p r c", r=G)[:, :, 0:NJ],
                        func=AF.Silu, bias=b1c, scale=1.0,
                    )
                    z2 = ps_mlp.tile([H, G * 512], F32, tag="mlp")
                    nc.tensor.matmul(z2[:, 0:512], lhsT=MLPV(w2T),
                                     rhs=MLPV(t1[:, 0:512]),
                                     start=True, stop=True)
                    nc.tensor.matmul(z2[:, 512:768], lhsT=MLPV(w2T),
                                     rhs=MLPV(t1[:, 512:768]),
                                     start=True, stop=True)
                    if slab is not None:
                        t2 = slab[:, g * (G * NJ):(g + 1) * (G * NJ)]
                    else:
                        t2 = work.tile([H, G * NJ], MLPDT, tag="t2")
                    nc.scalar.activation(
                        out=t2 if BF16MLP else t2.bitcast(F32R),
                        in_=z2[:, 0:G * NJ],
                        func=AF.Silu, bias=b2c, scale=1.0,
                    )
                    for r in range(G):
                        i = G * g + r
                        nc.tensor.matmul(
                            acc_ps,
                            lhsT=MLPV(winT[:, (NI - 1) - i:(2 * NI - 1) - i]),
                            rhs=MLPV(t2[:, r * NJ:(r + 1) * NJ]),
                            start=(i == 0), stop=(i == NI - 1),
                        )

                phi_ps = ps_sm.tile([H, NJ], F32, tag="sm")

                def coord_group(g):
                    emit_group(combC, cb1_sb[:, l:l + 1], c1hjT_w[:, l, :],
                               c2T_w[:, l, :], cb2_sb[:, l:l + 1],
                               c3w_w[:, l, :], phi_ps[0:NI, :], None, g)

                if not last:
                    att_ps = ps_sm.tile([H, NJ], F32, tag="sm")
                    m2slab = slabp.tile([H, NI * NJ], MLPDT, tag="m2")
                    # edge groups, with early coord groups stitched in to keep
                    # the PE fed during edge-phase ACT waits
                    NSTITCH = 0
                    for g in range(NGRP):
                        emit_group(combE, eb1_sb[:, l:l + 1], e1hjT_w[:, l, :],
                                   e2T_w[:, l, :], eb2_sb[:, l:l + 1],
                                   attw_w[:, l, :], att_ps[0:NI, :], m2slab, g)
                        if g >= NGRP - NSTITCH:
                            coord_group(g - (NGRP - NSTITCH))
                    # ---- C: gated message sum, stitched into coord groups ----
                    sg = cp1.tile([NI, NJ], F32, tag="sg")
                    nc.scalar.activation(out=sg, in_=att_ps[0:NI, :],
                                         func=AF.Sigmoid)
                    gmask = cp1.tile([NI, NJ], F32, tag="gmask")
                    nc.vector.tensor_tensor(out=gmask, in0=sg, in1=maskc_sb,
                                            op=OP.mult)
                    msumT = cp1.tile([H, NI], F32, tag="msumT")
                    if BF16MLP:
                        gmask_m = cp1.tile([NI, NJ], BF16, tag="gmb")
                        nc.vector.tensor_copy(gmask_m, gmask)
                    else:
                        gmask_m = gmask

                    def msum_chunk(i4):
                        growc = rowsp.tile([1, 2 * NJ], MLPDT, tag="growc")
                        nc.sync.dma_start(
                            out=growc[:] if BF16MLP else growc[:].bitcast(F32R),
                            in_=gmask_m[2 * i4:2 * (i4 + 1), :] if BF16MLP else
                            gmask_m[2 * i4:2 * (i4 + 1), :].bitcast(F32R))
                        for q in range(2):
                            i = 2 * i4 + q
                            gb = ps_sm.tile([H, NJ], F32, tag="sm")
                            nc.tensor.matmul(
                                gb, lhsT=MLPV(ones_w),
                                rhs=MLPV(growc[0:1, q * NJ:(q + 1) * NJ]),
                                start=True, stop=True)
                            mg = mgp.tile([H, NJ], F32, tag="mg")
                            nc.vector.scalar_tensor_tensor(
                                out=mg, in0=m2slab[:, i * NJ:(i + 1) * NJ],
                                scalar=1.0, in1=gb, op0=OP.mult, op1=OP.mult,
                                accum_out=msumT[:, i:i + 1])

                    # remaining coord groups with msum chunks stitched between
                    ncg = NGRP - NSTITCH
                    nch = NI // 2
                    cursor = 0
                    for k, g in enumerate(range(NSTITCH, NGRP)):
                        coord_group(g)
                        hi = (k + 1) * nch // ncg
                        while cursor < hi:
                            msum_chunk(cursor)
                            cursor += 1
                    if DEBUG and l == 0:
                        nc.sync.dma_start(out=dbg["gmask"][:], in_=gmask)
                        nc.sync.dma_start(out=dbg["msum"][:], in_=msumT)
                    nc.sync.dma_start(out=mag_in[l][:], in_=msumT)
                    nc.gpsimd.collective_compute(
                        "AllGather", OP.bypass, replica_groups=rg,
                        ins=[mag_in[l][:]], outs=[mag_out[l][:]],
                    )
                    msumF = cp1.tile([H, NJ], F32, tag="msumF")
                    for r in range(NC):
                        nc.sync.dma_start(
                            out=msumF[:, r * NI:(r + 1) * NI].bitcast(F32R),
                            in_=mag_out[l][r * H:(r + 1) * H, :].bitcast(F32R))
                    # node MLP (all 384 nodes, redundant on every core)
                    z1 = ps_sm.tile([H, NJ], F32, tag="sm")
                    RMM(z1, lhsT=nw1hT_sb[:, l, :], rhs=hT_l,
                        start=True, stop=False)
                    RMM(z1, lhsT=nw1mT_sb[:, l, :], rhs=msumF,
                        start=False, stop=True)
                    z1b = cp1.tile([H, NJ], F32, tag="z1b")
                    nc.vector.tensor_scalar(out=z1b, in0=z1,
                                            scalar1=nb1_sb[:, l:l + 1],
                                            scalar2=None, op0=OP.add)
                    sgn = cp1.tile([H, NJ], F32, tag="sgn")
                    nc.scalar.activation(out=sgn, in_=z1, func=AF.Sigmoid,
                                         bias=nb1_sb[:, l:l + 1], scale=1.0)
                    t1n = cp1.tile([H, NJ], F32, tag="t1n")
                    nc.vector.tensor_tensor(out=t1n[:].bitcast(F32R), in0=z1b,
                                            in1=sgn, op=OP.mult)
                    z2n = ps_sm.tile([H, NJ], F32, tag="sm")
                    RMM(z2n, lhsT=nw2T_sb[:, l, :], rhs=t1n,
                        start=True, stop=True)
                    h_T = cp2.tile([H, NJ], F32, tag="hT")
                    nc.vector.tensor_scalar(out=h_T[:].bitcast(F32R), in0=z2n,
                                            scalar1=nb2_sb[:, l:l + 1],
                                            scalar2=None, op0=OP.add)
                    # local copy of this core's own h rows for the next layer
                    z1m = ps_sm.tile([H, NI], F32, tag="sm")
                    nc.tensor.matmul(z1m, lhsT=nw1hT_sb[:, l, :], rhs=hmy_l,
                                     start=True, stop=False)
                    nc.tensor.matmul(z1m, lhsT=nw1mT_sb[:, l, :], rhs=msumT,
                                     start=False, stop=True)
                    z1bm = cp1.tile([H, NI], F32, tag="z1bm")
                    nc.vector.tensor_scalar(out=z1bm, in0=z1m,
                                            scalar1=nb1_sb[:, l:l + 1],
                                            scalar2=None, op0=OP.add)
                    sgnm = cp1.tile([H, NI], F32, tag="sgnm")
                    nc.scalar.activation(out=sgnm, in_=z1m, func=AF.Sigmoid,
                                         bias=nb1_sb[:, l:l + 1], scale=1.0)
                    t1nm = cp1.tile([H, NI], F32, tag="t1nm")
                    nc.vector.tensor_tensor(out=t1nm[:].bitcast(F32R), in0=z1bm,
                                            in1=sgnm, op=OP.mult)
                    z2m = ps_sm.tile([H, NI], F32, tag="sm")
                    RMM(z2m, lhsT=nw2T_sb[:, l, :], rhs=t1nm,
                        start=True, stop=True)
                    h_my = cp2.tile([H, NI], F32, tag="hmy")
                    nc.vector.tensor_scalar(out=h_my[:].bitcast(F32R), in0=z2m,
                                            scalar1=nb2_sb[:, l:l + 1],
                                            scalar2=None, op0=OP.add)
                else:
                    for g in range(NGRP):
                        coord_group(g)

                # ---- phi stream + x update ----
                phis = cp1.tile([NI, NJ], F32, tag="phis")
                nc.vector.tensor_scalar(out=phis, in0=phi_ps[0:NI, :],
                                        scalar1=cb3c_sb[:, l:l + 1], scalar2=None,
                                        op0=OP.add)
                s = cp1.tile([NI, NJ], F32, tag="s")
                nc.vector.tensor_tensor(out=s, in0=phis, in1=u, op=OP.mult)
                nc.vector.tensor_tensor(out=s, in0=s, in1=maskc_sb, op=OP.mult)
                xnew = cp2.tile([NI, D], F32, tag="xnew")
                for c in range(D):
                    xm = cp1.tile([NI, NJ], F32, tag="xm")
                    xcol = cp1.tile([NI, 1], F32, tag=f"xcol{c}")
                    nc.vector.scalar_tensor_tensor(
                        out=xm, in0=diff[c], scalar=1.0, in1=s,
                        op0=OP.mult, op1=OP.mult, accum_out=xcol)
                    nc.vector.tensor_tensor(out=xnew[:, c:c + 1], in0=xcol,
                                            in1=x_my[:, c:c + 1], op=OP.add)
                if DEBUG and l == 0:
                    nc.sync.dma_start(out=dbg["phis"][:], in_=phis)
                    nc.sync.dma_start(out=dbg["x1"][:], in_=xnew)
                    nc.sync.dma_start(out=dbg["h1"][:], in_=h_T)
                if not last:
                    nc.sync.dma_start(out=xag_in[l].rearrange("c n -> n c"),
                                      in_=xnew)
                else:
                    nc.sync.dma_start(out=xag_in[l][:], in_=xnew)
                nc.gpsimd.collective_compute(
                    "AllGather", OP.bypass, replica_groups=rg,
                    ins=[xag_in[l][:]], outs=[xag_out[l][:]],
                )
                if not last:
                    x_my = xnew
                else:
                    nc.sync.dma_start(out=o_x[:], in_=xag_out[l][:])

    nc.finalize()
    return nc


def _prep_inputs(inputs):
    """Host-side prep: per-core input maps from full arrays."""
    f = lambda a: np.ascontiguousarray(np.asarray(a), dtype=np.float32)
    x_inp = f(inputs["x_inp"])
    emb_w = f(inputs["emb_w"])
    emb_b = f(inputs["emb_b"])
    coord_w1 = f(inputs["coord_w1"])
    coord_b1 = f(inputs["coord_b1"])
    coord_w2 = f(inputs["coord_w2"])
    coord_b2 = f(inputs["coord_b2"])
    coord_w3 = f(inputs["coord_w3"])
    coord_b3 = f(inputs["coord_b3"])
    edge_w1 = f(inputs["edge_w1"])
    edge_b1 = f(inputs["edge_b1"])
    edge_w2 = f(inputs["edge_w2"])
    edge_b2 = f(inputs["edge_b2"])
    node_w1 = f(inputs["node_w1"])
    node_b1 = f(inputs["node_b1"])
    node_w2 = f(inputs["node_w2"])
    node_b2 = f(inputs["node_b2"])
    att_w = f(inputs["att_w"])

    x0 = x_inp.reshape(N, D)
    eye = np.eye(N, dtype=np.float32)

    def stackT(w, lo, hi):
        return np.ascontiguousarray(
            np.stack([w[l, :, lo:hi].T for l in range(w.shape[0])]))

    def win(w3):
        nl = w3.shape[0]
        out = np.zeros((nl, H, 2 * NI - 1), np.float32)
        out[:, :, NI - 1] = w3[:, 0, :]
        return out

    shared = dict(
        x0rows=np.ascontiguousarray(x0.T.reshape(1, D * N)),
        c1hiT=stackT(coord_w1, 0, H),
        c1hjT=stackT(coord_w1, H, 2 * H),
        c1drep=np.ascontiguousarray(np.tile(coord_w1[:, :, 2 * H], (1, 16))),
        cb1=np.ascontiguousarray(coord_b1.T),
        c2T=np.ascontiguousarray(np.stack([coord_w2[l].T for l in range(L)])),
        cb2=np.ascontiguousarray(coord_b2.T),
        c3w=win(coord_w3),
        cb3c=np.ascontiguousarray(
            np.broadcast_to(coord_b3[:, 0][None, :], (NI, L))),
        e1hiT=stackT(edge_w1, 0, H),
        e1hjT=stackT(edge_w1, H, 2 * H),
        e1drep=np.ascontiguousarray(np.tile(edge_w1[:, :, 2 * H], (1, 16))),
        ones6k=np.ones((1, 16 * NJ), np.float32),
        eb1=np.ascontiguousarray(edge_b1.T),
        e2T=np.ascontiguousarray(np.stack([edge_w2[l].T for l in range(L - 1)])),
        eb2=np.ascontiguousarray(edge_b2.T),
        attw=win(att_w),
        nw1hT=stackT(node_w1, 0, H),
        nw1mT=stackT(node_w1, H, 2 * H),
        nb1=np.ascontiguousarray(node_b1.T),
        nw2T=np.ascontiguousarray(np.stack([node_w2[l].T for l in range(L - 1)])),
        nb2=np.ascontiguousarray(node_b2.T),
        ones128=np.ones((1, H), np.float32),
    )
    in_maps = []
    for c in range(NC):
        m = dict(shared)
        m["embw"] = np.ascontiguousarray(
            emb_w[c * EMB_ROWS:(c + 1) * EMB_ROWS, :])
        m["embbT"] = np.ascontiguousarray(
            emb_b[c * EMB_ROWS:(c + 1) * EMB_ROWS].reshape(NI, H).T)
        m["x0my"] = np.ascontiguousarray(x0[c * NI:(c + 1) * NI, :])
        m["maskc"] = np.ascontiguousarray(1.0 - eye[c * NI:(c + 1) * NI, :])
        m["eyec"] = np.ascontiguousarray(eye[c * NI:(c + 1) * NI, :])
        in_maps.append(m)
    return in_maps


def _run(inputs, trace=False, **kw):
    from concourse.bass_utils import run_bass_kernel_spmd
    if "nc" not in _cache:
        _cache["nc"] = _build_nc()
    in_maps = _prep_inputs(inputs)
    return run_bass_kernel_spmd(_cache["nc"], in_maps, list(range(NC)),
                                trace=trace, **kw)


def kernel(**inputs) -> np.ndarray:
    res = _run(inputs)
    return np.asarray(res.results[0]["o_x"], dtype=np.float32).reshape(N * D)


# revision 34
# speedup vs baseline: 1.0082x; 1.0082x over previous
"""EGNN (N=384, D=3, H=128, L=4) Bass kernel for 8 TRN2 NeuronCores.

Sharding: rows of the N x N edge grid split across 8 cores (48 rows each).
Each core holds full h; per layer it computes its row-block of the coord/edge
MLPs and row-sums (phi-weighted coordinate update, gated message sum), then
AllGathers the per-node x updates and msum rows. The h-node MLP is computed
redundantly on every core (384 cols, tiny). The embedding row-sum of
emb_w (49152 x 384, the dominant HBM traffic) is sharded 1/8 per core.

Self-contained: shapes hardcoded, inputs are the full unsharded arrays.
"""
import os
import numpy as np

DEBUG = os.environ.get("KDEBUG", "0") == "1"
BF16MLP = os.environ.get("KBF16", "0") == "1"

N, D, H, L = 384, 3, 128, 4
NC = 8
NI = N // NC          # 48 rows per core
NJ = N                # 384 cols
G = 2                 # i-rows per group
NGRP = NI // G        # 24 groups
EMB_ROWS = N * H // NC  # 6144 emb_w rows per core

_cache = {}


def _build_nc():
    import concourse.bass as bass
    import concourse.bacc as bacc
    import concourse.tile as tile
    from concourse import mybir

    F32 = mybir.dt.float32
    AF = mybir.ActivationFunctionType
    OP = mybir.AluOpType

    nc = bacc.Bacc(None, target_bir_lowering=False)
    F32R = mybir.dt.float32r

    def RMM(out, lhsT, rhs, **kw):
        nc.tensor.matmul(out, lhsT=lhsT.bitcast(F32R), rhs=rhs.bitcast(F32R), **kw)

    def P(name, shape):
        return nc.declare_dram_parameter(name, list(shape), F32, isOutput=False)

    # per-core inputs
    embw = P("embw", (EMB_ROWS, NJ))
    embbT = P("embbT", (H, NI))
    x0my = P("x0my", (NI, D))
    maskc = P("maskc", (NI, NJ))
    eyec = P("eyec", (NI, NJ))
    # shared inputs
    x0rows = P("x0rows", (1, D * NJ))
    c1hiT = P("c1hiT", (L, H, H))
    c1hjT = P("c1hjT", (L, H, H))
    c1drep = P("c1drep", (L, 16 * H))
    cb1 = P("cb1", (H, L))
    c2T = P("c2T", (L, H, H))
    cb2 = P("cb2", (H, L))
    c3w = P("c3w", (L, H, 2 * NI - 1))
    cb3c = P("cb3c", (NI, L))
    e1hiT = P("e1hiT", (L - 1, H, H))
    e1hjT = P("e1hjT", (L - 1, H, H))
    e1drep = P("e1drep", (L - 1, 16 * H))
    ones6k = P("ones6k", (1, 16 * NJ))
    eb1 = P("eb1", (H, L - 1))
    e2T = P("e2T", (L - 1, H, H))
    eb2 = P("eb2", (H, L - 1))
    attw = P("attw", (L - 1, H, 2 * NI - 1))
    nw1hT = P("nw1hT", (L - 1, H, H))
    nw1mT = P("nw1mT", (L - 1, H, H))
    nb1 = P("nb1", (H, L - 1))
    nw2T = P("nw2T", (L - 1, H, H))
    nb2 = P("nb2", (H, L - 1))
    ones128 = P("ones128", (1, H))

    o_x = nc.declare_dram_parameter("o_x", [N, D], F32, isOutput=True)
    dbg = {}
    if DEBUG:
        for nm, shp in [("h0", (H, NJ)), ("d2", (NI, NJ)), ("u", (NI, NJ)),
                        ("t1c", (H, G * NJ)), ("phis", (NI, NJ)),
                        ("msum", (H, NI)), ("x1", (NI, D)), ("h1", (H, NJ)),
                        ("gmask", (NI, NJ))]:
            dbg[nm] = nc.declare_dram_parameter("dbg_" + nm, list(shp), F32,
                                                isOutput=True)

    # collective bounce buffers
    hag_in = nc.dram_tensor("hag_in", [H, NI], F32)
    hag_out = nc.dram_tensor("hag_out", [NC * H, NI], F32, addr_space="Shared")
    xag_in = [nc.dram_tensor(f"xag_in{l}", [D, NI], F32) for l in range(L - 1)]
    xag_in.append(nc.dram_tensor(f"xag_in3", [NI, D], F32))
    xag_out = [nc.dram_tensor(f"xag_out{l}", [NC, D, NI], F32, addr_space="Shared")
               for l in range(L - 1)]
    xag_out.append(nc.dram_tensor(f"xag_out3", [N, D], F32, addr_space="Shared"))
    mag_in = [nc.dram_tensor(f"mag_in{l}", [H, NI], F32) for l in range(L - 1)]
    mag_out = [nc.dram_tensor(f"mag_out{l}", [NC * H, NI], F32, addr_space="Shared")
               for l in range(L - 1)]
    rg = [list(range(NC))]

    with tile.TileContext(nc) as tc:
        with (
            tc.tile_pool(name="consts", bufs=1) as consts,
            tc.tile_pool(name="embp", bufs=2) as embp,
            tc.tile_pool(name="work", bufs=2) as work,
            tc.tile_pool(name="slab", bufs=1) as slabp,
            tc.tile_pool(name="cp1", bufs=1) as cp1,
            tc.tile_pool(name="cp2", bufs=2) as cp2,
            tc.tile_pool(name="rows", bufs=2) as rowsp,
            tc.tile_pool(name="mgp", bufs=1) as mgp,
            tc.tile_pool(name="ps_mlp", bufs=3, space="PSUM") as ps_mlp,
            tc.tile_pool(name="ps_sm", bufs=2, space="PSUM") as ps_sm,
        ):
            # ---- load constants ----
            def load(pname, ap_in, shape, rnd=False):
                t = consts.tile(list(shape), F32, tag=pname)
                o = t[:].bitcast(F32R) if rnd else t
                nc.sync.dma_start(out=o, in_=ap_in.bitcast(F32R) if rnd else ap_in)
                return t

            c1hiT_sb = load("c1hiT", c1hiT.rearrange("l p x -> p l x"), (H, L, H),
                            rnd=True)
            c1hjT_sb = load("c1hjT", c1hjT.rearrange("l p x -> p l x"), (H, L, H),
                            rnd=True)
            c2T_sb = load("c2T", c2T.rearrange("l p x -> p l x"), (H, L, H),
                          rnd=True)
            c3w_sb = load("c3w", c3w.rearrange("l p x -> p l x"),
                          (H, L, 2 * NI - 1), rnd=True)

            cb1_sb = load("cb1", cb1[:], (H, L))
            cb2_sb = load("cb2", cb2[:], (H, L))
            cb3c_sb = load("cb3c", cb3c[:], (NI, L))
            e1hiT_sb = load("e1hiT", e1hiT.rearrange("l p x -> p l x"),
                            (H, L - 1, H), rnd=True)
            e1hjT_sb = load("e1hjT", e1hjT.rearrange("l p x -> p l x"),
                            (H, L - 1, H), rnd=True)
            e2T_sb = load("e2T", e2T.rearrange("l p x -> p l x"), (H, L - 1, H),
                          rnd=True)
            attw_sb = load("attw", attw.rearrange("l p x -> p l x"),
                           (H, L - 1, 2 * NI - 1), rnd=True)

            eb1_sb = load("eb1", eb1[:], (H, L - 1))
            eb2_sb = load("eb2", eb2[:], (H, L - 1))
            nw1hT_sb = load("nw1hT", nw1hT.rearrange("l p x -> p l x"),
                            (H, L - 1, H), rnd=True)
            nw1mT_sb = load("nw1mT", nw1mT.rearrange("l p x -> p l x"),
                            (H, L - 1, H), rnd=True)
            nw2T_sb = load("nw2T", nw2T.rearrange("l p x -> p l x"),
                           (H, L - 1, H), rnd=True)
            nb1_sb = load("nb1", nb1[:], (H, L - 1))
            nb2_sb = load("nb2", nb2[:], (H, L - 1))
            maskc_sb = load("maskc", maskc[:], (NI, NJ))
            eyec_sb = load("eyec", eyec[:], (NI, NJ))
            embbT_sb = load("embbT", embbT[:], (H, NI))
            ones_sb = load("ones128", ones128[:], (1, H), rnd=True)
            x0my_sb = load("x0my", x0my[:], (NI, D))

            BF16 = mybir.dt.bfloat16
            MLPDT = BF16 if BF16MLP else F32
            def MLPV(ap):
                # view for matmul operands of MLP-pass tiles
                return ap if BF16MLP else ap.bitcast(F32R)

            if BF16MLP:
                def tobf(t, pname):
                    b = consts.tile(list(t.shape), BF16, tag=pname + "_bf")
                    nc.vector.tensor_copy(b, t)
                    return b
                c1hjT_w = tobf(c1hjT_sb, "c1hjT")
                c2T_w = tobf(c2T_sb, "c2T")
                c3w_w = tobf(c3w_sb, "c3w")
                e1hjT_w = tobf(e1hjT_sb, "e1hjT")
                e2T_w = tobf(e2T_sb, "e2T")
                attw_w = tobf(attw_sb, "attw")
                ones_w = tobf(ones_sb, "ones128")
            else:
                c1hjT_w, c2T_w = c1hjT_sb, c2T_sb
                c3w_w, e1hjT_w = c3w_sb, e1hjT_sb
                e2T_w, attw_w, ones_w = e2T_sb, attw_sb, ones_sb

            d2ones = consts.tile([66, 16 * NJ], F32, tag="d2ones")
            combC = consts.tile([66, 16 * H], F32, tag="combC")
            combE = consts.tile([66, 16 * H], F32, tag="combE")
            for k in range(3):
                nc.sync.dma_start(out=d2ones[32 * k + 1:32 * k + 2, :]
                                  .bitcast(F32R), in_=ones6k[:].bitcast(F32R))

            # ---- phase 0: embedding row-sum (memory bound) ----
            hT0 = cp1.tile([H, NI], F32, tag="hT0")
            EB = 4  # nodes per embedding DMA
            for t in range(NI // EB):
                et = embp.tile([H, EB, NJ], F32, tag="embt")
                nc.sync.dma_start(
                    out=et,
                    in_=embw[t * EB * H:(t + 1) * EB * H, :]
                    .rearrange("(a p) j -> p a j", p=H))
                with nc.allow_low_precision(reason="f32r storage, f32 accum"):
                    nc.vector.tensor_reduce(
                        out=hT0[:, t * EB:(t + 1) * EB].bitcast(F32R), in_=et,
                        axis=mybir.AxisListType.X, op=OP.add,
                    )
            nc.vector.tensor_tensor(out=hT0[:].bitcast(F32R), in0=hT0,
                                    in1=embbT_sb, op=OP.add)
            nc.sync.dma_start(out=hag_in[:], in_=hT0)
            nc.gpsimd.collective_compute(
                "AllGather", OP.bypass, replica_groups=rg,
                ins=[hag_in[:]], outs=[hag_out[:]],
            )
            h_T = cp2.tile([H, NJ], F32, tag="hT")
            for r in range(NC):
                nc.sync.dma_start(out=h_T[:, r * NI:(r + 1) * NI].bitcast(F32R),
                                  in_=hag_out[r * H:(r + 1) * H, :].bitcast(F32R))
            h_my = hT0  # this core's own h rows (h_T columns 48c..48c+48)
            if DEBUG:
                nc.sync.dma_start(out=dbg["h0"][:], in_=h_T)

            x_my = x0my_sb

            for l in range(L):
                last = l == L - 1
                # ---- A: coordinate prep ----
                diff = []
                for c in range(D):
                    xb = cp1.tile([NI, NJ], F32, tag=f"xb{c}")
                    if l == 0:
                        bsrc = bass.AP(tensor=x0rows, offset=c * NJ,
                                       ap=[[0, NI], [1, NJ]])
                    else:
                        bsrc = bass.AP(tensor=xag_out[l - 1], offset=c * NI,
                                       ap=[[0, NI], [D * NI, NC], [1, NI]])
                    nc.sync.dma_start(out=xb, in_=bsrc)
                    dc = cp2.tile([NI, NJ], F32, tag=f"diff{c}")
                    nc.vector.tensor_scalar(
                        out=dc, in0=xb, scalar1=x_my[:, c:c + 1], scalar2=None,
                        op0=OP.subtract,
                    )
                    diff.append(dc)
                d2 = cp1.tile([NI, NJ], F32, tag="d2")
                tmp = cp1.tile([NI, NJ], F32, tag="ctmp")
                nc.vector.tensor_tensor(out=d2, in0=diff[0], in1=diff[0], op=OP.mult)
                nc.vector.tensor_tensor(out=tmp, in0=diff[1], in1=diff[1], op=OP.mult)
                nc.vector.tensor_tensor(out=d2, in0=d2, in1=tmp, op=OP.add)
                nc.vector.tensor_tensor(out=tmp, in0=diff[2], in1=diff[2], op=OP.mult)
                nc.vector.tensor_tensor(out=d2, in0=d2, in1=tmp, op=OP.add)
                d2s = cp1.tile([NI, NJ], F32, tag="d2s")
                nc.vector.tensor_tensor(out=d2s, in0=d2, in1=eyec_sb, op=OP.add)
                dn = cp1.tile([NI, NJ], F32, tag="dn")
                nc.scalar.activation(out=dn, in_=d2s, func=AF.Sqrt)
                nc.vector.tensor_scalar(out=dn, in0=dn, scalar1=1.0, scalar2=None,
                                        op0=OP.add)
                u = cp2.tile([NI, NJ], F32, tag="u")
                nc.vector.reciprocal(out=u, in_=dn)

                if DEBUG and l == 0:
                    nc.sync.dma_start(out=dbg["d2"][:], in_=d2)
                    nc.sync.dma_start(out=dbg["u"][:], in_=u)

                # ---- B: edge-grid MLP groups ----
                hT_l = h_T    # h for THIS layer (h_T gets rebound by node MLP)
                hmy_l = h_my
                if BF16MLP:
                    hT_mlp = cp2.tile([H, NJ], BF16, tag="hTb")
                    nc.vector.tensor_copy(hT_mlp, h_T)
                    d2_mlp = cp1.tile([NI, NJ], BF16, tag="d2b")
                    nc.vector.tensor_copy(d2_mlp, d2)
                else:
                    hT_mlp = h_T
                    d2_mlp = d2
                for k in range(3):
                    nc.sync.dma_start(
                        out=d2ones[32 * k:32 * k + 1, :].bitcast(F32R),
                        in_=d2_mlp[16 * k:16 * (k + 1), :].bitcast(F32R))

                def make_comb(hiT, wdrep_l, comb, atag):
                    # A_T = (W1hi @ h_my).T via one matmul; stripe [wd; A_row]
                    # pairs into comb for the K=2 d-pass
                    At_ps = ps_sm.tile([NI, H], F32, tag="sm")
                    RMM(At_ps, lhsT=hmy_l, rhs=hiT, start=True, stop=True)
                    At = cp1.tile([NI, H], F32, tag=atag)
                    nc.vector.tensor_copy(At[:].bitcast(F32R), At_ps)
                    for k in range(3):
                        nc.sync.dma_start(
                            out=comb[32 * k:32 * k + 1, :].bitcast(F32R),
                            in_=wdrep_l.bitcast(F32R))
                        nc.sync.dma_start(
                            out=comb[32 * k + 1:32 * k + 2, :].bitcast(F32R),
                            in_=At[16 * k:16 * (k + 1), :].bitcast(F32R))

                make_comb(c1hiT_sb[:, l, :], c1drep[l:l + 1, :], combC, "Atc")
                if not last:
                    make_comb(e1hiT_sb[:, l, :], e1drep[l:l + 1, :], combE, "Ate")

                def emit_group(comb, b1c, hjT, w2T, b2c, winT, acc_ps,
                               slab, g):
                    pre = ps_mlp.tile([H, G * 512], F32, tag="mlp")

                    def mm_hj(first):
                        for r in range(G):
                            nc.tensor.matmul(
                                pre[:, r * 512:r * 512 + NJ],
                                lhsT=MLPV(hjT),
                                rhs=MLPV(hT_mlp[:, :]),
                                start=first, stop=not first)

                    def mm_d(first):
                        for r in range(G):
                            i = G * g + r
                            kq, iq = i // 16, i % 16
                            nc.tensor.matmul(
                                pre[:, r * 512:r * 512 + NJ],
                                lhsT=comb[32 * kq:32 * kq + 2,
                                          iq * H:(iq + 1) * H].bitcast(F32R),
                                rhs=d2ones[32 * kq:32 * kq + 2,
                                           iq * NJ:(iq + 1) * NJ].bitcast(F32R),
                                start=first, stop=not first)

                    mm_hj(True)
                    mm_d(False)
                    t1 = work.tile([H, G * NJ], MLPDT, tag="t1")
                    nc.scalar.activation(
                        out=(t1[:, :] if BF16MLP else t1[:, :].bitcast(F32R))
                        .rearrange("p (r c) -> p r c", r=G),
                        in_=pre.rearrange("p (r c) -> p r c", r=G)[:, :, 0:NJ],
                        func=AF.Silu, bias=b1c, scale=1.0,
                    )
                    z2 = ps_mlp.tile([H, G * 512], F32, tag="mlp")
                    nc.tensor.matmul(z2[:, 0:512], lhsT=MLPV(w2T),
                                     rhs=MLPV(t1[:, 0:512]),
                                     start=True, stop=True)
                    nc.tensor.matmul(z2[:, 512:768], lhsT=MLPV(w2T),
                                     rhs=MLPV(t1[:, 512:768]),
                                     start=True, stop=True)
                    if slab is not None:
                        t2 = slab[:, g * (G * NJ):(g + 1) * (G * NJ)]
                    else:
                        t2 = work.tile([H, G * NJ], MLPDT, tag="t2")
                    nc.scalar.activation(
                        out=t2 if BF16MLP else t2.bitcast(F32R),
                        in_=z2[:, 0:G * NJ],
                        func=AF.Silu, bias=b2c, scale=1.0,
                    )
                    if acc_ps is not None:
                        for r in range(G):
                            i = G * g + r
                            nc.tensor.matmul(
                                acc_ps,
                                lhsT=MLPV(winT[:, (NI - 1) - i:(2 * NI - 1) - i]),
                                rhs=MLPV(t2[:, r * NJ:(r + 1) * NJ]),
                                start=(i == 0), stop=(i == NI - 1),
                            )

                phi_ps = ps_sm.tile([H, NJ], F32, tag="sm")

                def coord_group(g):
                    emit_group(combC, cb1_sb[:, l:l + 1], c1hjT_w[:, l, :],
                               c2T_w[:, l, :], cb2_sb[:, l:l + 1],
                               c3w_w[:, l, :], phi_ps[0:NI, :], None, g)

                if not last:
                    att_ps = ps_sm.tile([H, NJ], F32, tag="sm")
                    m2slab = slabp.tile([H, NI * NJ], MLPDT, tag="m2")
                    # edge groups, with early coord groups stitched in to keep
                    # the PE fed during edge-phase ACT waits
                    for g in range(NGRP):
                        emit_group(combE, eb1_sb[:, l:l + 1], e1hjT_w[:, l, :],
                                   e2T_w[:, l, :], eb2_sb[:, l:l + 1],
                                   attw_w[:, l, :], None, m2slab, g)
                    for i in range(NI):
                        nc.tensor.matmul(
                            att_ps[0:NI, :],
                            lhsT=MLPV(attw_w[:, l, (NI - 1) - i:(2 * NI - 1) - i]),
                            rhs=MLPV(m2slab[:, i * NJ:(i + 1) * NJ]),
                            start=(i == 0), stop=(i == NI - 1),
                        )
                    # ---- C: gated message sum, stitched into coord groups ----
                    sg = cp1.tile([NI, NJ], F32, tag="sg")
                    nc.scalar.activation(out=sg, in_=att_ps[0:NI, :],
                                         func=AF.Sigmoid)
                    gmask = cp1.tile([NI, NJ], F32, tag="gmask")
                    nc.vector.tensor_tensor(out=gmask, in0=sg, in1=maskc_sb,
                                            op=OP.mult)
                    msumT = cp1.tile([H, NI], F32, tag="msumT")
                    if BF16MLP:
                        gmask_m = cp1.tile([NI, NJ], BF16, tag="gmb")
                        nc.vector.tensor_copy(gmask_m, gmask)
                    else:
                        gmask_m = gmask

                    def msum_chunk(i4):
                        growc = rowsp.tile([1, 2 * NJ], MLPDT, tag="growc")
                        nc.sync.dma_start(
                            out=growc[:] if BF16MLP else growc[:].bitcast(F32R),
                            in_=gmask_m[2 * i4:2 * (i4 + 1), :] if BF16MLP else
                            gmask_m[2 * i4:2 * (i4 + 1), :].bitcast(F32R))
                        for q in range(2):
                            i = 2 * i4 + q
                            gb = ps_sm.tile([H, NJ], F32, tag="sm")
                            nc.tensor.matmul(
                                gb, lhsT=MLPV(ones_w),
                                rhs=MLPV(growc[0:1, q * NJ:(q + 1) * NJ]),
                                start=True, stop=True)
                            mg = mgp.tile([H, NJ], F32, tag="mg")
                            nc.vector.scalar_tensor_tensor(
                                out=mg, in0=m2slab[:, i * NJ:(i + 1) * NJ],
                                scalar=1.0, in1=gb, op0=OP.mult, op1=OP.mult,
                                accum_out=msumT[:, i:i + 1])

                    # remaining coord groups with msum chunks stitched between
                    ncg = NGRP
                    nch = NI // 2
                    cursor = 0
                    for k, g in enumerate(range(NGRP)):
                        coord_group(g)
                        hi = (k + 1) * nch // ncg
                        while cursor < hi:
                            msum_chunk(cursor)
                            cursor += 1
                    if DEBUG and l == 0:
                        nc.sync.dma_start(out=dbg["gmask"][:], in_=gmask)
                        nc.sync.dma_start(out=dbg["msum"][:], in_=msumT)
                    nc.sync.dma_start(out=mag_in[l][:], in_=msumT)
                    nc.gpsimd.collective_compute(
                        "AllGather", OP.bypass, replica_groups=rg,
                        ins=[mag_in[l][:]], outs=[mag_out[l][:]],
                    )
                    msumF = cp1.tile([H, NJ], F32, tag="msumF")
                    for r in range(NC):
                        nc.sync.dma_start(
                            out=msumF[:, r * NI:(r + 1) * NI].bitcast(F32R),
                            in_=mag_out[l][r * H:(r + 1) * H, :].bitcast(F32R))
                    # node MLP (all 384 nodes, redundant on every core)
                    z1 = ps_sm.tile([H, NJ], F32, tag="sm")
                    RMM(z1, lhsT=nw1hT_sb[:, l, :], rhs=hT_l,
                        start=True, stop=False)
                    RMM(z1, lhsT=nw1mT_sb[:, l, :], rhs=msumF,
                        start=False, stop=True)
                    z1b = cp1.tile([H, NJ], F32, tag="z1b")
                    nc.vector.tensor_scalar(out=z1b, in0=z1,
                                            scalar1=nb1_sb[:, l:l + 1],
                                            scalar2=None, op0=OP.add)
                    sgn = cp1.tile([H, NJ], F32, tag="sgn")
                    nc.scalar.activation(out=sgn, in_=z1, func=AF.Sigmoid,
                                         bias=nb1_sb[:, l:l + 1], scale=1.0)
                    t1n = cp1.tile([H, NJ], F32, tag="t1n")
                    nc.vector.tensor_tensor(out=t1n[:].bitcast(F32R), in0=z1b,
                                            in1=sgn, op=OP.mult)
                    z2n = ps_sm.tile([H, NJ], F32, tag="sm")
                    RMM(z2n, lhsT=nw2T_sb[:, l, :], rhs=t1n,
                        start=True, stop=True)
                    h_T = cp2.tile([H, NJ], F32, tag="hT")
                    nc.vector.tensor_scalar(out=h_T[:].bitcast(F32R), in0=z2n,
                                            scalar1=nb2_sb[:, l:l + 1],
                                            scalar2=None, op0=OP.add)
                    # local copy of this core's own h rows for the next layer
                    z1m = ps_sm.tile([H, NI], F32, tag="sm")
                    nc.tensor.matmul(z1m, lhsT=nw1hT_sb[:, l, :], rhs=hmy_l,
                                     start=True, stop=False)
                    nc.tensor.matmul(z1m, lhsT=nw1mT_sb[:, l, :], rhs=msumT,
                                     start=False, stop=True)
                    z1bm = cp1.tile([H, NI], F32, tag="z1bm")
                    nc.vector.tensor_scalar(out=z1bm, in0=z1m,
                                            scalar1=nb1_sb[:, l:l + 1],
                                            scalar2=None, op0=OP.add)
                    sgnm = cp1.tile([H, NI], F32, tag="sgnm")
                    nc.scalar.activation(out=sgnm, in_=z1m, func=AF.Sigmoid,
                                         bias=nb1_sb[:, l:l + 1], scale=1.0)
                    t1nm = cp1.tile([H, NI], F32, tag="t1nm")
                    nc.vector.tensor_tensor(out=t1nm[:].bitcast(F32R), in0=z1bm,
                                            in1=sgnm, op=OP.mult)
                    z2m = ps_sm.tile([H, NI], F32, tag="sm")
                    RMM(z2m, lhsT=nw2T_sb[:, l, :], rhs=t1nm,
                        start=True, stop=True)
                    h_my = cp2.tile([H, NI], F32, tag="hmy")
                    nc.vector.tensor_scalar(out=h_my[:].bitcast(F32R), in0=z2m,
                                            scalar1=nb2_sb[:, l:l + 1],
                                            scalar2=None, op0=OP.add)
                else:
                    for g in range(NGRP):
                        coord_group(g)

                # ---- phi stream + x update ----
                phis = cp1.tile([NI, NJ], F32, tag="phis")
                nc.vector.tensor_scalar(out=phis, in0=phi_ps[0:NI, :],
                                        scalar1=cb3c_sb[:, l:l + 1], scalar2=None,
                                        op0=OP.add)
                s = cp1.tile([NI, NJ], F32, tag="s")
                nc.vector.tensor_tensor(out=s, in0=phis, in1=u, op=OP.mult)
                nc.vector.tensor_tensor(out=s, in0=s, in1=maskc_sb, op=OP.mult)
                xnew = cp2.tile([NI, D], F32, tag="xnew")
                for c in range(D):
                    xm = cp1.tile([NI, NJ], F32, tag="xm")
                    xcol = cp1.tile([NI, 1], F32, tag=f"xcol{c}")
                    nc.vector.scalar_tensor_tensor(
                        out=xm, in0=diff[c], scalar=1.0, in1=s,
                        op0=OP.mult, op1=OP.mult, accum_out=xcol)
                    nc.vector.tensor_tensor(out=xnew[:, c:c + 1], in0=xcol,
                                            in1=x_my[:, c:c + 1], op=OP.add)
                if DEBUG and l == 0:
                    nc.sync.dma_start(out=dbg["phis"][:], in_=phis)
                    nc.sync.dma_start(out=dbg["x1"][:], in_=xnew)
                    nc.sync.dma_start(out=dbg["h1"][:], in_=h_T)
                if not last:
                    nc.sync.dma_start(out=xag_in[l].rearrange("c n -> n c"),
                                      in_=xnew)
                else:
                    nc.sync.dma_start(out=xag_in[l][:], in_=xnew)
                nc.gpsimd.collective_compute(
                    "AllGather", OP.bypass, replica_groups=rg,
                    ins=[xag_in[l][:]], outs=[xag_out[l][:]],
                )
                if not last:
                    x_my = xnew
                else:
                    nc.sync.dma_start(out=o_x[:], in_=xag_out[l][:])

    nc.finalize()
    return nc


def _prep_inputs(inputs):
    """Host-side prep: per-core input maps from full arrays."""
    f = lambda a: np.ascontiguousarray(np.asarray(a), dtype=np.float32)
    x_inp = f(inputs["x_inp"])
    emb_w = f(inputs["emb_w"])
    emb_b = f(inputs["emb_b"])
    coord_w1 = f(inputs["coord_w1"])
    coord_b1 = f(inputs["coord_b1"])
    coord_w2 = f(inputs["coord_w2"])
    coord_b2 = f(inputs["coord_b2"])
    coord_w3 = f(inputs["coord_w3"])
    coord_b3 = f(inputs["coord_b3"])
    edge_w1 = f(inputs["edge_w1"])
    edge_b1 = f(inputs["edge_b1"])
    edge_w2 = f(inputs["edge_w2"])
    edge_b2 = f(inputs["edge_b2"])
    node_w1 = f(inputs["node_w1"])
    node_b1 = f(inputs["node_b1"])
    node_w2 = f(inputs["node_w2"])
    node_b2 = f(inputs["node_b2"])
    att_w = f(inputs["att_w"])

    x0 = x_inp.reshape(N, D)
    eye = np.eye(N, dtype=np.float32)

    def stackT(w, lo, hi):
        return np.ascontiguousarray(
            np.stack([w[l, :, lo:hi].T for l in range(w.shape[0])]))

    def win(w3):
        nl = w3.shape[0]
        out = np.zeros((nl, H, 2 * NI - 1), np.float32)
        out[:, :, NI - 1] = w3[:, 0, :]
        return out

    shared = dict(
        x0rows=np.ascontiguousarray(x0.T.reshape(1, D * N)),
        c1hiT=stackT(coord_w1, 0, H),
        c1hjT=stackT(coord_w1, H, 2 * H),
        c1drep=np.ascontiguousarray(np.tile(coord_w1[:, :, 2 * H], (1, 16))),
        cb1=np.ascontiguousarray(coord_b1.T),
        c2T=np.ascontiguousarray(np.stack([coord_w2[l].T for l in range(L)])),
        cb2=np.ascontiguousarray(coord_b2.T),
        c3w=win(coord_w3),
        cb3c=np.ascontiguousarray(
            np.broadcast_to(coord_b3[:, 0][None, :], (NI, L))),
        e1hiT=stackT(edge_w1, 0, H),
        e1hjT=stackT(edge_w1, H, 2 * H),
        e1drep=np.ascontiguousarray(np.tile(edge_w1[:, :, 2 * H], (1, 16))),
        ones6k=np.ones((1, 16 * NJ), np.float32),
        eb1=np.ascontiguousarray(edge_b1.T),
        e2T=np.ascontiguousarray(np.stack([edge_w2[l].T for l in range(L - 1)])),
        eb2=np.ascontiguousarray(edge_b2.T),
        attw=win(att_w),
        nw1hT=stackT(node_w1, 0, H),
        nw1mT=stackT(node_w1, H, 2 * H),
        nb1=np.ascontiguousarray(node_b1.T),
        nw2T=np.ascontiguousarray(np.stack([node_w2[l].T for l in range(L - 1)])),
        nb2=np.ascontiguousarray(node_b2.T),
        ones128=np.ones((1, H), np.float32),
    )
    in_maps = []
    for c in range(NC):
        m = dict(shared)
        m["embw"] = np.ascontiguousarray(
            emb_w[c * EMB_ROWS:(c + 1) * EMB_ROWS, :])
        m["embbT"] = np.ascontiguousarray(
            emb_b[c * EMB_ROWS:(c + 1) * EMB_ROWS].reshape(NI, H).T)
        m["x0my"] = np.ascontiguousarray(x0[c * NI:(c + 1) * NI, :])
        m["maskc"] = np.ascontiguousarray(1.0 - eye[c * NI:(c + 1) * NI, :])
        m["eyec"] = np.ascontiguousarray(eye[c * NI:(c + 1) * NI, :])
        in_maps.append(m)
    return in_maps


def _run(inputs, trace=False, **kw):
    from concourse.bass_utils import run_bass_kernel_spmd
    if "nc" not in _cache:
        _cache["nc"] = _build_nc()
    in_maps = _prep_inputs(inputs)
    return run_bass_kernel_spmd(_cache["nc"], in_maps, list(range(NC)),
                                trace=trace, **kw)


def kernel(**inputs) -> np.ndarray:
    res = _run(inputs)
    return np.asarray(res.results[0]["o_x"], dtype=np.float32).reshape(N * D)


# revision 35
# speedup vs baseline: 1.0168x; 1.0085x over previous
"""EGNN (N=384, D=3, H=128, L=4) Bass kernel for 8 TRN2 NeuronCores.

Sharding: rows of the N x N edge grid split across 8 cores (48 rows each).
Each core holds full h; per layer it computes its row-block of the coord/edge
MLPs and row-sums (phi-weighted coordinate update, gated message sum), then
AllGathers the per-node x updates and msum rows. The h-node MLP is computed
redundantly on every core (384 cols, tiny). The embedding row-sum of
emb_w (49152 x 384, the dominant HBM traffic) is sharded 1/8 per core.

Self-contained: shapes hardcoded, inputs are the full unsharded arrays.
"""
import os
import numpy as np

DEBUG = os.environ.get("KDEBUG", "0") == "1"
BF16MLP = os.environ.get("KBF16", "0") == "1"

N, D, H, L = 384, 3, 128, 4
NC = 8
NI = N // NC          # 48 rows per core
NJ = N                # 384 cols
G = 2                 # i-rows per group
NGRP = NI // G        # 24 groups
EMB_ROWS = N * H // NC  # 6144 emb_w rows per core

_cache = {}


def _build_nc():
    import concourse.bass as bass
    import concourse.bacc as bacc
    import concourse.tile as tile
    from concourse import mybir

    F32 = mybir.dt.float32
    AF = mybir.ActivationFunctionType
    OP = mybir.AluOpType

    nc = bacc.Bacc(None, target_bir_lowering=False)
    F32R = mybir.dt.float32r

    def RMM(out, lhsT, rhs, **kw):
        nc.tensor.matmul(out, lhsT=lhsT.bitcast(F32R), rhs=rhs.bitcast(F32R), **kw)

    def P(name, shape):
        return nc.declare_dram_parameter(name, list(shape), F32, isOutput=False)

    # per-core inputs
    embw = P("embw", (EMB_ROWS, NJ))
    embbT = P("embbT", (H, NI))
    x0my = P("x0my", (NI, D))
    maskc = P("maskc", (NI, NJ))
    eyec = P("eyec", (NI, NJ))
    # shared inputs
    x0rows = P("x0rows", (1, D * NJ))
    c1hiT = P("c1hiT", (L, H, H))
    c1hjT = P("c1hjT", (L, H, H))
    c1drep = P("c1drep", (L, 16 * H))
    cb1 = P("cb1", (H, L))
    c2T = P("c2T", (L, H, H))
    cb2 = P("cb2", (H, L))
    c3w = P("c3w", (L, H, 2 * NI - 1))
    cb3c = P("cb3c", (NI, L))
    e1hiT = P("e1hiT", (L - 1, H, H))
    e1hjT = P("e1hjT", (L - 1, H, H))
    e1drep = P("e1drep", (L - 1, 16 * H))
    ones6k = P("ones6k", (1, 16 * NJ))
    eb1 = P("eb1", (H, L - 1))
    e2T = P("e2T", (L - 1, H, H))
    eb2 = P("eb2", (H, L - 1))
    attw = P("attw", (L - 1, H, 2 * NI - 1))
    nw1hT = P("nw1hT", (L - 1, H, H))
    nw1mT = P("nw1mT", (L - 1, H, H))
    nb1 = P("nb1", (H, L - 1))
    nw2T = P("nw2T", (L - 1, H, H))
    nb2 = P("nb2", (H, L - 1))
    ones128 = P("ones128", (1, H))

    o_x = nc.declare_dram_parameter("o_x", [N, D], F32, isOutput=True)
    dbg = {}
    if DEBUG:
        for nm, shp in [("h0", (H, NJ)), ("d2", (NI, NJ)), ("u", (NI, NJ)),
                        ("t1c", (H, G * NJ)), ("phis", (NI, NJ)),
                        ("msum", (H, NI)), ("x1", (NI, D)), ("h1", (H, NJ)),
                        ("gmask", (NI, NJ))]:
            dbg[nm] = nc.declare_dram_parameter("dbg_" + nm, list(shp), F32,
                                                isOutput=True)

    # collective bounce buffers
    hag_in = nc.dram_tensor("hag_in", [H, NI], F32)
    hag_out = nc.dram_tensor("hag_out", [NC * H, NI], F32, addr_space="Shared")
    xag_in = [nc.dram_tensor(f"xag_in{l}", [D, NI], F32) for l in range(L - 1)]
    xag_in.append(nc.dram_tensor(f"xag_in3", [NI, D], F32))
    xag_out = [nc.dram_tensor(f"xag_out{l}", [NC, D, NI], F32, addr_space="Shared")
               for l in range(L - 1)]
    xag_out.append(nc.dram_tensor(f"xag_out3", [N, D], F32, addr_space="Shared"))
    mag_in = [nc.dram_tensor(f"mag_in{l}", [H, NI], F32) for l in range(L - 1)]
    mag_out = [nc.dram_tensor(f"mag_out{l}", [NC * H, NI], F32, addr_space="Shared")
               for l in range(L - 1)]
    rg = [list(range(NC))]

    with tile.TileContext(nc) as tc:
        with (
            tc.tile_pool(name="consts", bufs=1) as consts,
            tc.tile_pool(name="embp", bufs=2) as embp,
            tc.tile_pool(name="work", bufs=2) as work,
            tc.tile_pool(name="slab", bufs=1) as slabp,
            tc.tile_pool(name="cp1", bufs=1) as cp1,
            tc.tile_pool(name="cp2", bufs=2) as cp2,
            tc.tile_pool(name="rows", bufs=2) as rowsp,
            tc.tile_pool(name="mgp", bufs=1) as mgp,
            tc.tile_pool(name="ps_mlp", bufs=3, space="PSUM") as ps_mlp,
            tc.tile_pool(name="ps_sm", bufs=2, space="PSUM") as ps_sm,
        ):
            # ---- load constants ----
            def load(pname, ap_in, shape, rnd=False):
                t = consts.tile(list(shape), F32, tag=pname)
                o = t[:].bitcast(F32R) if rnd else t
                nc.sync.dma_start(out=o, in_=ap_in.bitcast(F32R) if rnd else ap_in)
                return t

            c1hiT_sb = load("c1hiT", c1hiT.rearrange("l p x -> p l x"), (H, L, H),
                            rnd=True)
            c1hjT_sb = load("c1hjT", c1hjT.rearrange("l p x -> p l x"), (H, L, H),
                            rnd=True)
            c2T_sb = load("c2T", c2T.rearrange("l p x -> p l x"), (H, L, H),
                          rnd=True)
            c3w_sb = load("c3w", c3w.rearrange("l p x -> p l x"),
                          (H, L, 2 * NI - 1), rnd=True)

            cb1_sb = load("cb1", cb1[:], (H, L))
            cb2_sb = load("cb2", cb2[:], (H, L))
            cb3c_sb = load("cb3c", cb3c[:], (NI, L))
            e1hiT_sb = load("e1hiT", e1hiT.rearrange("l p x -> p l x"),
                            (H, L - 1, H), rnd=True)
            e1hjT_sb = load("e1hjT", e1hjT.rearrange("l p x -> p l x"),
                            (H, L - 1, H), rnd=True)
            e2T_sb = load("e2T", e2T.rearrange("l p x -> p l x"), (H, L - 1, H),
                          rnd=True)
            attw_sb = load("attw", attw.rearrange("l p x -> p l x"),
                           (H, L - 1, 2 * NI - 1), rnd=True)

            eb1_sb = load("eb1", eb1[:], (H, L - 1))
            eb2_sb = load("eb2", eb2[:], (H, L - 1))
            nw1hT_sb = load("nw1hT", nw1hT.rearrange("l p x -> p l x"),
                            (H, L - 1, H), rnd=True)
            nw1mT_sb = load("nw1mT", nw1mT.rearrange("l p x -> p l x"),
                            (H, L - 1, H), rnd=True)
            nw2T_sb = load("nw2T", nw2T.rearrange("l p x -> p l x"),
                           (H, L - 1, H), rnd=True)
            nb1_sb = load("nb1", nb1[:], (H, L - 1))
            nb2_sb = load("nb2", nb2[:], (H, L - 1))
            maskc_sb = load("maskc", maskc[:], (NI, NJ))
            eyec_sb = load("eyec", eyec[:], (NI, NJ))
            embbT_sb = load("embbT", embbT[:], (H, NI))
            ones_sb = load("ones128", ones128[:], (1, H), rnd=True)
            x0my_sb = load("x0my", x0my[:], (NI, D))

            BF16 = mybir.dt.bfloat16
            MLPDT = BF16 if BF16MLP else F32
            def MLPV(ap):
                # view for matmul operands of MLP-pass tiles
                return ap if BF16MLP else ap.bitcast(F32R)

            if BF16MLP:
                def tobf(t, pname):
                    b = consts.tile(list(t.shape), BF16, tag=pname + "_bf")
                    nc.vector.tensor_copy(b, t)
                    return b
                c1hjT_w = tobf(c1hjT_sb, "c1hjT")
                c2T_w = tobf(c2T_sb, "c2T")
                c3w_w = tobf(c3w_sb, "c3w")
                e1hjT_w = tobf(e1hjT_sb, "e1hjT")
                e2T_w = tobf(e2T_sb, "e2T")
                attw_w = tobf(attw_sb, "attw")
                ones_w = tobf(ones_sb, "ones128")
            else:
                c1hjT_w, c2T_w = c1hjT_sb, c2T_sb
                c3w_w, e1hjT_w = c3w_sb, e1hjT_sb
                e2T_w, attw_w, ones_w = e2T_sb, attw_sb, ones_sb

            d2ones = consts.tile([66, 16 * NJ], F32, tag="d2ones")
            combC = consts.tile([66, 16 * H], F32, tag="combC")
            combE = consts.tile([66, 16 * H], F32, tag="combE")
            for k in range(3):
                nc.sync.dma_start(out=d2ones[32 * k + 1:32 * k + 2, :]
                                  .bitcast(F32R), in_=ones6k[:].bitcast(F32R))

            # ---- phase 0: embedding row-sum (memory bound) ----
            hT0 = cp1.tile([H, NI], F32, tag="hT0")
            EB = 4  # nodes per embedding DMA
            for t in range(NI // EB):
                et = embp.tile([H, EB, NJ], F32, tag="embt")
                eng = nc.sync if t % 2 == 0 else nc.scalar
                eng.dma_start(
                    out=et,
                    in_=embw[t * EB * H:(t + 1) * EB * H, :]
                    .rearrange("(a p) j -> p a j", p=H))
                with nc.allow_low_precision(reason="f32r storage, f32 accum"):
                    nc.vector.tensor_reduce(
                        out=hT0[:, t * EB:(t + 1) * EB].bitcast(F32R), in_=et,
                        axis=mybir.AxisListType.X, op=OP.add,
                    )
            nc.vector.tensor_tensor(out=hT0[:].bitcast(F32R), in0=hT0,
                                    in1=embbT_sb, op=OP.add)
            nc.sync.dma_start(out=hag_in[:], in_=hT0)
            nc.gpsimd.collective_compute(
                "AllGather", OP.bypass, replica_groups=rg,
                ins=[hag_in[:]], outs=[hag_out[:]],
            )
            h_T = cp2.tile([H, NJ], F32, tag="hT")
            for r in range(NC):
                nc.sync.dma_start(out=h_T[:, r * NI:(r + 1) * NI].bitcast(F32R),
                                  in_=hag_out[r * H:(r + 1) * H, :].bitcast(F32R))
            h_my = hT0  # this core's own h rows (h_T columns 48c..48c+48)
            if DEBUG:
                nc.sync.dma_start(out=dbg["h0"][:], in_=h_T)

            x_my = x0my_sb

            for l in range(L):
                last = l == L - 1
                # ---- A: coordinate prep ----
                diff = []
                for c in range(D):
                    xb = cp1.tile([NI, NJ], F32, tag=f"xb{c}")
                    if l == 0:
                        bsrc = bass.AP(tensor=x0rows, offset=c * NJ,
                                       ap=[[0, NI], [1, NJ]])
                    else:
                        bsrc = bass.AP(tensor=xag_out[l - 1], offset=c * NI,
                                       ap=[[0, NI], [D * NI, NC], [1, NI]])
                    nc.sync.dma_start(out=xb, in_=bsrc)
                    dc = cp2.tile([NI, NJ], F32, tag=f"diff{c}")
                    nc.vector.tensor_scalar(
                        out=dc, in0=xb, scalar1=x_my[:, c:c + 1], scalar2=None,
                        op0=OP.subtract,
                    )
                    diff.append(dc)
                d2 = cp1.tile([NI, NJ], F32, tag="d2")
                tmp = cp1.tile([NI, NJ], F32, tag="ctmp")
                nc.vector.tensor_tensor(out=d2, in0=diff[0], in1=diff[0], op=OP.mult)
                nc.vector.tensor_tensor(out=tmp, in0=diff[1], in1=diff[1], op=OP.mult)
                nc.vector.tensor_tensor(out=d2, in0=d2, in1=tmp, op=OP.add)
                nc.vector.tensor_tensor(out=tmp, in0=diff[2], in1=diff[2], op=OP.mult)
                nc.vector.tensor_tensor(out=d2, in0=d2, in1=tmp, op=OP.add)
                d2s = cp1.tile([NI, NJ], F32, tag="d2s")
                nc.vector.tensor_tensor(out=d2s, in0=d2, in1=eyec_sb, op=OP.add)
                dn = cp1.tile([NI, NJ], F32, tag="dn")
                nc.scalar.activation(out=dn, in_=d2s, func=AF.Sqrt)
                nc.vector.tensor_scalar(out=dn, in0=dn, scalar1=1.0, scalar2=None,
                                        op0=OP.add)
                u = cp2.tile([NI, NJ], F32, tag="u")
                nc.vector.reciprocal(out=u, in_=dn)

                if DEBUG and l == 0:
                    nc.sync.dma_start(out=dbg["d2"][:], in_=d2)
                    nc.sync.dma_start(out=dbg["u"][:], in_=u)

                # ---- B: edge-grid MLP groups ----
                hT_l = h_T    # h for THIS layer (h_T gets rebound by node MLP)
                hmy_l = h_my
                if BF16MLP:
                    hT_mlp = cp2.tile([H, NJ], BF16, tag="hTb")
                    nc.vector.tensor_copy(hT_mlp, h_T)
                    d2_mlp = cp1.tile([NI, NJ], BF16, tag="d2b")
                    nc.vector.tensor_copy(d2_mlp, d2)
                else:
                    hT_mlp = h_T
                    d2_mlp = d2
                for k in range(3):
                    nc.sync.dma_start(
                        out=d2ones[32 * k:32 * k + 1, :].bitcast(F32R),
                        in_=d2_mlp[16 * k:16 * (k + 1), :].bitcast(F32R))

                def make_comb(hiT, wdrep_l, comb, atag):
                    # A_T = (W1hi @ h_my).T via one matmul; stripe [wd; A_row]
                    # pairs into comb for the K=2 d-pass
                    At_ps = ps_sm.tile([NI, H], F32, tag="sm")
                    RMM(At_ps, lhsT=hmy_l, rhs=hiT, start=True, stop=True)
                    At = cp1.tile([NI, H], F32, tag=atag)
                    nc.vector.tensor_copy(At[:].bitcast(F32R), At_ps)
                    for k in range(3):
                        nc.sync.dma_start(
                            out=comb[32 * k:32 * k + 1, :].bitcast(F32R),
                            in_=wdrep_l.bitcast(F32R))
                        nc.sync.dma_start(
                            out=comb[32 * k + 1:32 * k + 2, :].bitcast(F32R),
                            in_=At[16 * k:16 * (k + 1), :].bitcast(F32R))

                make_comb(c1hiT_sb[:, l, :], c1drep[l:l + 1, :], combC, "Atc")
                if not last:
                    make_comb(e1hiT_sb[:, l, :], e1drep[l:l + 1, :], combE, "Ate")

                def emit_group(comb, b1c, hjT, w2T, b2c, winT, acc_ps,
                               slab, g):
                    pre = ps_mlp.tile([H, G * 512], F32, tag="mlp")

                    def mm_hj(first):
                        for r in range(G):
                            nc.tensor.matmul(
                                pre[:, r * 512:r * 512 + NJ],
                                lhsT=MLPV(hjT),
                                rhs=MLPV(hT_mlp[:, :]),
                                start=first, stop=not first)

                    def mm_d(first):
                        for r in range(G):
                            i = G * g + r
                            kq, iq = i // 16, i % 16
                            nc.tensor.matmul(
                                pre[:, r * 512:r * 512 + NJ],
                                lhsT=comb[32 * kq:32 * kq + 2,
                                          iq * H:(iq + 1) * H].bitcast(F32R),
                                rhs=d2ones[32 * kq:32 * kq + 2,
                                           iq * NJ:(iq + 1) * NJ].bitcast(F32R),
                                start=first, stop=not first)

                    mm_hj(True)
                    mm_d(False)
                    t1 = work.tile([H, G * NJ], MLPDT, tag="t1")
                    nc.scalar.activation(
                        out=(t1[:, :] if BF16MLP else t1[:, :].bitcast(F32R))
                        .rearrange("p (r c) -> p r c", r=G),
                        in_=pre.rearrange("p (r c) -> p r c", r=G)[:, :, 0:NJ],
                        func=AF.Silu, bias=b1c, scale=1.0,
                    )
                    z2 = ps_mlp.tile([H, G * 512], F32, tag="mlp")
                    nc.tensor.matmul(z2[:, 0:512], lhsT=MLPV(w2T),
                                     rhs=MLPV(t1[:, 0:512]),
                                     start=True, stop=True)
                    nc.tensor.matmul(z2[:, 512:768], lhsT=MLPV(w2T),
                                     rhs=MLPV(t1[:, 512:768]),
                                     start=True, stop=True)
                    if slab is not None:
                        t2 = slab[:, g * (G * NJ):(g + 1) * (G * NJ)]
                    else:
                        t2 = work.tile([H, G * NJ], MLPDT, tag="t2")
                    nc.scalar.activation(
                        out=t2 if BF16MLP else t2.bitcast(F32R),
                        in_=z2[:, 0:G * NJ],
                        func=AF.Silu, bias=b2c, scale=1.0,
                    )
                    if acc_ps is not None:
                        pend.append((t2, g))
                        if len(pend) > 1:
                            flush_acc(pend.pop(0), acc_ps, winT)

                pend = []

                def flush_acc(item, acc_ps, winT):
                    t2p, gp = item
                    for r in range(G):
                        i = G * gp + r
                        nc.tensor.matmul(
                            acc_ps,
                            lhsT=MLPV(winT[:, (NI - 1) - i:(2 * NI - 1) - i]),
                            rhs=MLPV(t2p[:, r * NJ:(r + 1) * NJ]),
                            start=(i == 0), stop=(i == NI - 1),
                        )

                phi_ps = ps_sm.tile([H, NJ], F32, tag="sm")

                def coord_group(g):
                    emit_group(combC, cb1_sb[:, l:l + 1], c1hjT_w[:, l, :],
                               c2T_w[:, l, :], cb2_sb[:, l:l + 1],
                               c3w_w[:, l, :], phi_ps[0:NI, :], None, g)

                if not last:
                    att_ps = ps_sm.tile([H, NJ], F32, tag="sm")
                    m2slab = slabp.tile([H, NI * NJ], MLPDT, tag="m2")
                    # edge groups, with early coord groups stitched in to keep
                    # the PE fed during edge-phase ACT waits
                    for g in range(NGRP):
                        emit_group(combE, eb1_sb[:, l:l + 1], e1hjT_w[:, l, :],
                                   e2T_w[:, l, :], eb2_sb[:, l:l + 1],
                                   attw_w[:, l, :], None, m2slab, g)
                    for i in range(NI):
                        nc.tensor.matmul(
                            att_ps[0:NI, :],
                            lhsT=MLPV(attw_w[:, l, (NI - 1) - i:(2 * NI - 1) - i]),
                            rhs=MLPV(m2slab[:, i * NJ:(i + 1) * NJ]),
                            start=(i == 0), stop=(i == NI - 1),
                        )
                    # ---- C: gated message sum, stitched into coord groups ----
                    sg = cp1.tile([NI, NJ], F32, tag="sg")
                    nc.scalar.activation(out=sg, in_=att_ps[0:NI, :],
                                         func=AF.Sigmoid)
                    gmask = cp1.tile([NI, NJ], F32, tag="gmask")
                    nc.vector.tensor_tensor(out=gmask, in0=sg, in1=maskc_sb,
                                            op=OP.mult)
                    msumT = cp1.tile([H, NI], F32, tag="msumT")
                    if BF16MLP:
                        gmask_m = cp1.tile([NI, NJ], BF16, tag="gmb")
                        nc.vector.tensor_copy(gmask_m, gmask)
                    else:
                        gmask_m = gmask

                    def msum_chunk(i4):
                        growc = rowsp.tile([1, 2 * NJ], MLPDT, tag="growc")
                        nc.sync.dma_start(
                            out=growc[:] if BF16MLP else growc[:].bitcast(F32R),
                            in_=gmask_m[2 * i4:2 * (i4 + 1), :] if BF16MLP else
                            gmask_m[2 * i4:2 * (i4 + 1), :].bitcast(F32R))
                        for q in range(2):
                            i = 2 * i4 + q
                            gb = ps_sm.tile([H, NJ], F32, tag="sm")
                            nc.tensor.matmul(
                                gb, lhsT=MLPV(ones_w),
                                rhs=MLPV(growc[0:1, q * NJ:(q + 1) * NJ]),
                                start=True, stop=True)
                            mg = mgp.tile([H, NJ], F32, tag="mg")
                            nc.vector.scalar_tensor_tensor(
                                out=mg, in0=m2slab[:, i * NJ:(i + 1) * NJ],
                                scalar=1.0, in1=gb, op0=OP.mult, op1=OP.mult,
                                accum_out=msumT[:, i:i + 1])

                    # remaining coord groups with msum chunks stitched between
                    ncg = NGRP
                    nch = NI // 2
                    cursor = 0
                    for k, g in enumerate(range(NGRP)):
                        coord_group(g)
                        hi = (k + 1) * nch // ncg
                        while cursor < hi:
                            msum_chunk(cursor)
                            cursor += 1
                    while pend:
                        flush_acc(pend.pop(0), phi_ps[0:NI, :], c3w_w[:, l, :])
                    if DEBUG and l == 0:
                        nc.sync.dma_start(out=dbg["gmask"][:], in_=gmask)
                        nc.sync.dma_start(out=dbg["msum"][:], in_=msumT)
                    nc.sync.dma_start(out=mag_in[l][:], in_=msumT)
                    nc.gpsimd.collective_compute(
                        "AllGather", OP.bypass, replica_groups=rg,
                        ins=[mag_in[l][:]], outs=[mag_out[l][:]],
                    )
                    msumF = cp1.tile([H, NJ], F32, tag="msumF")
                    for r in range(NC):
                        nc.sync.dma_start(
                            out=msumF[:, r * NI:(r + 1) * NI].bitcast(F32R),
                            in_=mag_out[l][r * H:(r + 1) * H, :].bitcast(F32R))
                    # node MLP (all 384 nodes, redundant on every core)
                    z1 = ps_sm.tile([H, NJ], F32, tag="sm")
                    RMM(z1, lhsT=nw1hT_sb[:, l, :], rhs=hT_l,
                        start=True, stop=False)
                    RMM(z1, lhsT=nw1mT_sb[:, l, :], rhs=msumF,
                        start=False, stop=True)
                    z1b = cp1.tile([H, NJ], F32, tag="z1b")
                    nc.vector.tensor_scalar(out=z1b, in0=z1,
                                            scalar1=nb1_sb[:, l:l + 1],
                                            scalar2=None, op0=OP.add)
                    sgn = cp1.tile([H, NJ], F32, tag="sgn")
                    nc.scalar.activation(out=sgn, in_=z1, func=AF.Sigmoid,
                                         bias=nb1_sb[:, l:l + 1], scale=1.0)
                    t1n = cp1.tile([H, NJ], F32, tag="t1n")
                    nc.vector.tensor_tensor(out=t1n[:].bitcast(F32R), in0=z1b,
                                            in1=sgn, op=OP.mult)
                    z2n = ps_sm.tile([H, NJ], F32, tag="sm")
                    RMM(z2n, lhsT=nw2T_sb[:, l, :], rhs=t1n,
                        start=True, stop=True)
                    h_T = cp2.tile([H, NJ], F32, tag="hT")
                    nc.vector.tensor_scalar(out=h_T[:].bitcast(F32R), in0=z2n,
                                            scalar1=nb2_sb[:, l:l + 1],
                                            scalar2=None, op0=OP.add)
                    # local copy of this core's own h rows for the next layer
                    z1m = ps_sm.tile([H, NI], F32, tag="sm")
                    nc.tensor.matmul(z1m, lhsT=nw1hT_sb[:, l, :], rhs=hmy_l,
                                     start=True, stop=False)
                    nc.tensor.matmul(z1m, lhsT=nw1mT_sb[:, l, :], rhs=msumT,
                                     start=False, stop=True)
                    z1bm = cp1.tile([H, NI], F32, tag="z1bm")
                    nc.vector.tensor_scalar(out=z1bm, in0=z1m,
                                            scalar1=nb1_sb[:, l:l + 1],
                                            scalar2=None, op0=OP.add)
                    sgnm = cp1.tile([H, NI], F32, tag="sgnm")
                    nc.scalar.activation(out=sgnm, in_=z1m, func=AF.Sigmoid,
                                         bias=nb1_sb[:, l:l + 1], scale=1.0)
                    t1nm = cp1.tile([H, NI], F32, tag="t1nm")
                    nc.vector.tensor_tensor(out=t1nm[:].bitcast(F32R), in0=z1bm,
                                            in1=sgnm, op=OP.mult)
                    z2m = ps_sm.tile([H, NI], F32, tag="sm")
                    RMM(z2m, lhsT=nw2T_sb[:, l, :], rhs=t1nm,
                        start=True, stop=True)
                    h_my = cp2.tile([H, NI], F32, tag="hmy")
                    nc.vector.tensor_scalar(out=h_my[:].bitcast(F32R), in0=z2m,
                                            scalar1=nb2_sb[:, l:l + 1],
                                            scalar2=None, op0=OP.add)
                else:
                    for g in range(NGRP):
                        coord_group(g)
                    while pend:
                        flush_acc(pend.pop(0), phi_ps[0:NI, :], c3w_w[:, l, :])

                # ---- phi stream + x update ----
                phis = cp1.tile([NI, NJ], F32, tag="phis")
                nc.vector.tensor_scalar(out=phis, in0=phi_ps[0:NI, :],
                                        scalar1=cb3c_sb[:, l:l + 1], scalar2=None,
                                        op0=OP.add)
                s = cp1.tile([NI, NJ], F32, tag="s")
                nc.vector.tensor_tensor(out=s, in0=phis, in1=u, op=OP.mult)
                nc.vector.tensor_tensor(out=s, in0=s, in1=maskc_sb, op=OP.mult)
                xnew = cp2.tile([NI, D], F32, tag="xnew")
                for c in range(D):
                    xm = cp1.tile([NI, NJ], F32, tag="xm")
                    xcol = cp1.tile([NI, 1], F32, tag=f"xcol{c}")
                    nc.vector.scalar_tensor_tensor(
                        out=xm, in0=diff[c], scalar=1.0, in1=s,
                        op0=OP.mult, op1=OP.mult, accum_out=xcol)
                    nc.vector.tensor_tensor(out=xnew[:, c:c + 1], in0=xcol,
                                            in1=x_my[:, c:c + 1], op=OP.add)
                if DEBUG and l == 0:
                    nc.sync.dma_start(out=dbg["phis"][:], in_=phis)
                    nc.sync.dma_start(out=dbg["x1"][:], in_=xnew)
                    nc.sync.dma_start(out=dbg["h1"][:], in_=h_T)
                if not last:
                    nc.sync.dma_start(out=xag_in[l].rearrange("c n -> n c"),
                                      in_=xnew)
                else:
                    nc.sync.dma_start(out=xag_in[l][:], in_=xnew)
                nc.gpsimd.collective_compute(
                    "AllGather", OP.bypass, replica_groups=rg,
                    ins=[xag_in[l][:]], outs=[xag_out[l][:]],
                )
                if not last:
                    x_my = xnew
                else:
                    nc.sync.dma_start(out=o_x[:], in_=xag_out[l][:])

    nc.finalize()
    return nc


def _prep_inputs(inputs):
    """Host-side prep: per-core input maps from full arrays."""
    f = lambda a: np.ascontiguousarray(np.asarray(a), dtype=np.float32)
    x_inp = f(inputs["x_inp"])
    emb_w = f(inputs["emb_w"])
    emb_b = f(inputs["emb_b"])
    coord_w1 = f(inputs["coord_w1"])
    coord_b1 = f(inputs["coord_b1"])
    coord_w2 = f(inputs["coord_w2"])
    coord_b2 = f(inputs["coord_b2"])
    coord_w3 = f(inputs["coord_w3"])
    coord_b3 = f(inputs["coord_b3"])
    edge_w1 = f(inputs["edge_w1"])
    edge_b1 = f(inputs["edge_b1"])
    edge_w2 = f(inputs["edge_w2"])
    edge_b2 = f(inputs["edge_b2"])
    node_w1 = f(inputs["node_w1"])
    node_b1 = f(inputs["node_b1"])
    node_w2 = f(inputs["node_w2"])
    node_b2 = f(inputs["node_b2"])
    att_w = f(inputs["att_w"])

    x0 = x_inp.reshape(N, D)
    eye = np.eye(N, dtype=np.float32)

    def stackT(w, lo, hi):
        return np.ascontiguousarray(
            np.stack([w[l, :, lo:hi].T for l in range(w.shape[0])]))

    def win(w3):
        nl = w3.shape[0]
        out = np.zeros((nl, H, 2 * NI - 1), np.float32)
        out[:, :, NI - 1] = w3[:, 0, :]
        return out

    shared = dict(
        x0rows=np.ascontiguousarray(x0.T.reshape(1, D * N)),
        c1hiT=stackT(coord_w1, 0, H),
        c1hjT=stackT(coord_w1, H, 2 * H),
        c1drep=np.ascontiguousarray(np.tile(coord_w1[:, :, 2 * H], (1, 16))),
        cb1=np.ascontiguousarray(coord_b1.T),
        c2T=np.ascontiguousarray(np.stack([coord_w2[l].T for l in range(L)])),
        cb2=np.ascontiguousarray(coord_b2.T),
        c3w=win(coord_w3),
        cb3c=np.ascontiguousarray(
            np.broadcast_to(coord_b3[:, 0][None, :], (NI, L))),
        e1hiT=stackT(edge_w1, 0, H),
        e1hjT=stackT(edge_w1, H, 2 * H),
        e1drep=np.ascontiguousarray(np.tile(edge_w1[:, :, 2 * H], (1, 16))),
        ones6k=np.ones((1, 16 * NJ), np.float32),
        eb1=np.ascontiguousarray(edge_b1.T),
        e2T=np.ascontiguousarray(np.stack([edge_w2[l].T for l in range(L - 1)])),
        eb2=np.ascontiguousarray(edge_b2.T),
        attw=win(att_w),
        nw1hT=stackT(node_w1, 0, H),
        nw1mT=stackT(node_w1, H, 2 * H),
        nb1=np.ascontiguousarray(node_b1.T),
        nw2T=np.ascontiguousarray(np.stack([node_w2[l].T for l in range(L - 1)])),
        nb2=np.ascontiguousarray(node_b2.T),
        ones128=np.ones((1, H), np.float32),
    )
    in_maps = []
    for c in range(NC):
        m = dict(shared)
        m["embw"] = np.ascontiguousarray(
            emb_w[c * EMB_ROWS:(c + 1) * EMB_ROWS, :])
        m["embbT"] = np.ascontiguousarray(
            emb_b[c * EMB_ROWS:(c + 1) * EMB_ROWS].reshape(NI, H).T)
        m["x0my"] = np.ascontiguousarray(x0[c * NI:(c + 1) * NI, :])
        m["maskc"] = np.ascontiguousarray(1.0 - eye[c * NI:(c + 1) * NI, :])
        m["eyec"] = np.ascontiguousarray(eye[c * NI:(c + 1) * NI, :])
        in_maps.append(m)
    return in_maps


def _run(inputs, trace=False, **kw):
    from concourse.bass_utils import run_bass_kernel_spmd
    if "nc" not in _cache:
        _cache["nc"] = _build_nc()
    in_maps = _prep_inputs(inputs)
    return run_bass_kernel_spmd(_cache["nc"], in_maps, list(range(NC)),
                                trace=trace, **kw)


def kernel(**inputs) -> np.ndarray:
    res = _run(inputs)
    return np.asarray(res.results[0]["o_x"], dtype=np.float32).reshape(N * D)
